# revision 5
# baseline (speedup 1.0000x reference)
"""Trainium2 Bass kernel for nn_Net_31044023615490 (x-vector style net).

Pipeline (per NeuronCore, data-parallel over 8 segments of 1024 rows each):
  - 4-layer BN-MLP computed in transposed layout (features on partitions),
    bf16 matmuls with fp32 PSUM accumulation, BN+bias+ReLU fused into the
    scalar-engine activation (per-partition scale folded into weights on host).
  - Attention logits A^T = Wa^T @ h4T, softmax over time on the free axis.
  - DMA-xbar transposes bring As and h4 into time-on-partition layout for the
    pooled = As^T @ h4 and G = As^T @ As contractions over time.
  - Per-segment x mean/sumsq stats on the vector engine.
Host does the tiny [64 x 4608] head + log_softmax + penalty reduction in f64.
"""
from contextlib import ExitStack

import numpy as np
import ml_dtypes

N, D, H, R, O, B, L = 65536, 256, 512, 8, 4, 64, 1024
EPS = 1e-5
NCORES = 8
SEG = B // NCORES          # segments per core
ROWS = SEG * L             # rows per core
BF = ml_dtypes.bfloat16

TRACE = False              # test harness may flip this for profiling
LAST_RESULT = None         # BassKernelResults of the last run (when TRACE)

_BUILT = None              # cached (nc, names)


def _build_nc():
    import concourse.bass as bass
    import concourse.mybir as mybir
    import concourse.tile as tile
    from concourse import bacc
    from concourse.bass import ts

    dt = mybir.dt
    f32 = dt.float32
    bf16 = dt.bfloat16
    Alu = mybir.AluOpType
    Act = mybir.ActivationFunctionType

    nc = bacc.Bacc("TRN2", target_bir_lowering=False, debug=False)

    xT_d = nc.dram_tensor("xT", [128, 2, ROWS], bf16, kind="ExternalInput")
    w1_d = nc.dram_tensor("w1", [128, 2, 512], bf16, kind="ExternalInput")
    w2_d = nc.dram_tensor("w2", [128, 4, 512], bf16, kind="ExternalInput")
    w3_d = nc.dram_tensor("w3", [128, 4, 512], bf16, kind="ExternalInput")
    w4_d = nc.dram_tensor("w4", [128, 4, 512], bf16, kind="ExternalInput")
    wa_d = nc.dram_tensor("wa", [128, 4, 8], bf16, kind="ExternalInput")
    bias_d = nc.dram_tensor("bias", [128, 4, 4], f32, kind="ExternalInput")

    pooled_d = nc.dram_tensor("pooled", [SEG, 8, 512], f32, kind="ExternalOutput")
    g_d = nc.dram_tensor("gmat", [SEG, 8, 8], f32, kind="ExternalOutput")
    xsum_d = nc.dram_tensor("xsum", [128, 2, SEG], f32, kind="ExternalOutput")
    xsq_d = nc.dram_tensor("xsq", [128, 2, SEG], f32, kind="ExternalOutput")

    xT = xT_d.ap()

    with tile.TileContext(nc) as tc, ExitStack() as ctx:
        wpool = ctx.enter_context(tc.tile_pool(name="w", bufs=1))
        xpool = ctx.enter_context(tc.tile_pool(name="x", bufs=2))
        hpools = [ctx.enter_context(tc.tile_pool(name=f"h{i}", bufs=2))
                  for i in range(1, 5)]
        h4npool = ctx.enter_context(tc.tile_pool(name="h4n", bufs=2))
        aspool = ctx.enter_context(tc.tile_pool(name="ast", bufs=2))
        smpool = ctx.enter_context(tc.tile_pool(name="sm", bufs=2))
        scrpool = ctx.enter_context(tc.tile_pool(name="scr", bufs=2))
        stpool = ctx.enter_context(tc.tile_pool(name="st", bufs=2))
        statpool = ctx.enter_context(tc.tile_pool(name="stat", bufs=1))
        pmpool = ctx.enter_context(
            tc.tile_pool(name="pm", bufs=4, space=bass.MemorySpace.PSUM))
        papool = ctx.enter_context(
            tc.tile_pool(name="pa", bufs=2, space=bass.MemorySpace.PSUM))
        pppool = ctx.enter_context(
            tc.tile_pool(name="pp", bufs=1, space=bass.MemorySpace.PSUM))
        pgpool = ctx.enter_context(
            tc.tile_pool(name="pg", bufs=1, space=bass.MemorySpace.PSUM))

        # --- load weights/constants once ---
        w1 = wpool.tile([128, 2, 512], bf16)
        nc.gpsimd.dma_start(w1[:], w1_d.ap()[:])
        ws = [w1]
        for wd in (w2_d, w3_d, w4_d):
            w = wpool.tile([128, 4, 512], bf16, tag=wd.name)
            nc.gpsimd.dma_start(w[:], wd.ap()[:])
            ws.append(w)
        wa = wpool.tile([128, 4, 8], bf16)
        nc.gpsimd.dma_start(wa[:], wa_d.ap()[:])
        bias = wpool.tile([128, 4, 4], f32)
        nc.gpsimd.dma_start(bias[:], bias_d.ap()[:])

        xsum_acc = statpool.tile([128, 2, SEG], f32)
        xsq_acc = statpool.tile([128, 2, SEG], f32)

        for s in range(SEG):
            # --- input tile ---
            xt = xpool.tile([128, 2, L], bf16)
            nc.gpsimd.dma_start(xt[:], xT[:, :, ts(s, L)])

            # --- x stats (DVE) ---
            for ch in range(2):
                nc.vector.reduce_sum(xsum_acc[:, ch, s:s + 1], xt[:, ch, :],
                                     axis=mybir.AxisListType.X)
                scr = scrpool.tile([128, L], bf16)
                nc.scalar.activation(scr[:], xt[:, ch, :], Act.Square,
                                     accum_out=xsq_acc[:, ch, s:s + 1])

            # --- MLP in transposed layout ---
            prev, prev_k = xt, 2
            h = None
            for li in range(4):
                h = hpools[li].tile([128, 4, L], bf16)
                for tb in range(2):
                    for m in range(4):
                        ps = pmpool.tile([128, 512], f32)
                        for k in range(prev_k):
                            nc.tensor.matmul(
                                ps[:],
                                ws[li][:, k, ts(m, 128)],
                                prev[:, k, ts(tb, 512)],
                                start=(k == 0), stop=(k == prev_k - 1))
                        if li % 2 == 0:
                            # BN+ReLU on DVE: (psum + bias) max 0 -> bf16
                            nc.vector.tensor_scalar(
                                h[:, m, ts(tb, 512)], ps[:],
                                bias[:, li, m:m + 1], 0.0,
                                op0=Alu.add, op1=Alu.max)
                        else:
                            nc.scalar.activation(
                                h[:, m, ts(tb, 512)], ps[:], Act.Relu,
                                bias=bias[:, li, m:m + 1], scale=1.0)
                prev, prev_k = h, 4
            h4 = h

            # --- attention logits + softmax over time (free axis) ---
            amax = smpool.tile([8, 2], f32)
            easT = smpool.tile([8, L], f32)
            zpart = smpool.tile([8, 2], f32)
            pa_tiles = []
            for tb in range(2):
                pa = papool.tile([8, 512], f32)
                for k in range(4):
                    nc.tensor.matmul(pa[:], wa[:, k, :], h4[:, k, ts(tb, 512)],
                                     start=(k == 0), stop=(k == 3))
                nc.vector.reduce_max(amax[:, tb:tb + 1], pa[:],
                                     axis=mybir.AxisListType.X, negate=True)
                pa_tiles.append(pa)
            negmax = smpool.tile([8, 1], f32)
            nc.vector.tensor_tensor(negmax[:], amax[:, 0:1], amax[:, 1:2],
                                    op=Alu.min)
            for tb in range(2):
                nc.scalar.activation(easT[:, ts(tb, 512)], pa_tiles[tb][:],
                                     Act.Exp, bias=negmax[:, 0:1], scale=1.0,
                                     accum_out=zpart[:, tb:tb + 1])
            rz = smpool.tile([8, 1], f32)
            nc.vector.tensor_tensor(rz[:], zpart[:, 0:1], zpart[:, 1:2],
                                    op=Alu.add)
            nc.vector.reciprocal(rz[:], rz[:])
            asT16 = smpool.tile([16, L], bf16)
            nc.vector.memset(asT16[:], 0.0)
            nc.vector.tensor_scalar_mul(asT16[0:8, :], easT[:], rz[:, 0:1])

            # --- transposes to time-on-partition layout (DMA xbar) ---
            as_t = aspool.tile([128, 8, 16], bf16)
            nc.sync.dma_start(as_t[:], asT16[:], transpose=True)
            h4n = h4npool.tile([128, 8, 4, 128], bf16)
            for ch in range(4):
                nc.sync.dma_start(h4n[:, :, ch, :], h4[:, ch, :],
                                  transpose=True)

            # --- pooled and G (contract over time) ---
            pp = pppool.tile([8, 512], f32)
            for tt in range(8):
                nc.tensor.matmul(pp[:], as_t[:, tt, 0:8], h4n[:, tt, :, :],
                                 start=(tt == 0), stop=(tt == 7))
            pg = pgpool.tile([8, 8], f32)
            for tt in range(8):
                nc.tensor.matmul(pg[:], as_t[:, tt, 0:8], as_t[:, tt, 0:8],
                                 start=(tt == 0), stop=(tt == 7))

            post = stpool.tile([8, 512], f32)
            nc.scalar.copy(post[:], pp[:])
            nc.gpsimd.dma_start(pooled_d.ap()[s], post[:])
            gst = stpool.tile([8, 8], f32)
            nc.vector.tensor_copy(gst[:], pg[:])
            nc.gpsimd.dma_start(g_d.ap()[s], gst[:])

        nc.gpsimd.dma_start(xsum_d.ap()[:], xsum_acc[:])
        nc.gpsimd.dma_start(xsq_d.ap()[:], xsq_acc[:])

    nc.compile()
    return nc


def _get_nc():
    global _BUILT
    if _BUILT is None:
        _BUILT = _build_nc()
    return _BUILT


def _fuse_bn(W, b, g, be, m, v):
    scale = (np.asarray(g, np.float32)
             / np.sqrt(np.asarray(v, np.float32) + EPS))
    Wf = np.asarray(W, np.float32) * scale[None, :]
    bias = (np.asarray(b, np.float32) - np.asarray(m, np.float32)) * scale \
        + np.asarray(be, np.float32)
    return Wf, bias


def _chunked(Wf):
    """[K, F] f32 -> [128, K//128, F] bf16 (partition-major k-chunks)."""
    K, F = Wf.shape
    return np.ascontiguousarray(
        Wf.reshape(K // 128, 128, F).transpose(1, 0, 2)).astype(BF)


def kernel(**inputs):
    global LAST_RESULT
    from concourse.bass_utils import run_bass_kernel_spmd

    x = np.asarray(inputs["x"], np.float32)

    biases = []
    wts = []
    for i in range(1, 5):
        Wf, bias = _fuse_bn(inputs[f"W{i}"], inputs[f"b{i}"], inputs[f"g{i}"],
                            inputs[f"be{i}"], inputs[f"m{i}"], inputs[f"v{i}"])
        wts.append(_chunked(Wf))
        biases.append(bias)
    # bias[p, layer, m] = bias_layer[m*128 + p]
    bias_t = np.ascontiguousarray(
        np.stack([b.reshape(4, 128).T for b in biases], axis=1)
    ).astype(np.float32)
    wa_t = _chunked(np.asarray(inputs["Wa"], np.float32))

    shared = {"w1": wts[0], "w2": wts[1], "w3": wts[2], "w4": wts[3],
              "wa": wa_t, "bias": bias_t}

    in_maps = []
    for c in range(NCORES):
        xc = x[c * ROWS:(c + 1) * ROWS, :]                 # [ROWS, 256]
        xct = np.ascontiguousarray(
            xc.T.reshape(2, 128, ROWS).transpose(1, 0, 2)).astype(BF)
        in_maps.append({"xT": xct, **shared})

    nc = _get_nc()
    res = run_bass_kernel_spmd(nc, in_maps, core_ids=list(range(NCORES)),
                               trace=TRACE)
    LAST_RESULT = res

    pooled = np.zeros((B, R * H), np.float64)
    G = np.zeros((B, R, R), np.float64)
    xsum = np.zeros((B, D), np.float64)
    xsq = np.zeros((B, D), np.float64)
    for c, out in enumerate(res.results):
        sl = slice(c * SEG, (c + 1) * SEG)
        pooled[sl] = out["pooled"].astype(np.float64).reshape(SEG, R * H)
        G[sl] = out["gmat"].astype(np.float64)
        # xsum/xsq: [128, 2, SEG] -> [SEG, 256] with d = ch*128 + p
        xs = out["xsum"].astype(np.float64).transpose(2, 1, 0).reshape(SEG, D)
        xq = out["xsq"].astype(np.float64).transpose(2, 1, 0).reshape(SEG, D)
        xsum[sl] = xs
        xsq[sl] = xq

    penalty = np.sum((G - 1.0) ** 2)

    mean = xsum / L
    var = (xsq - L * mean * mean) / (L - 1)
    std = np.sqrt(np.maximum(var, 0.0))
    feat = np.concatenate([pooled, mean, std], axis=1)

    c64 = {k: np.asarray(inputs[k], np.float64)
           for k in ("Wo1", "bo1", "go", "beo", "mo", "vo", "Wo2", "bo2")}
    z = feat @ c64["Wo1"] + c64["bo1"]
    o = np.maximum(
        c64["go"] * (z - c64["mo"]) / np.sqrt(c64["vo"] + EPS) + c64["beo"],
        0.0)
    logits = o @ c64["Wo2"] + c64["bo2"]
    ls = logits - logits.max(axis=1, keepdims=True)
    logp = (ls - np.log(np.exp(ls).sum(axis=1, keepdims=True)))

    return (logp.astype(np.float32), np.float32(penalty))


# revision 8
# speedup vs baseline: 1.1092x; 1.1092x over previous
"""Trainium2 Bass kernel for nn_Net_31044023615490 (x-vector style net).

Pipeline (per NeuronCore, data-parallel over 8 segments of 1024 rows each):
  - 4-layer BN-MLP computed in transposed layout (features on partitions),
    bf16 matmuls with fp32 PSUM accumulation, BN+bias+ReLU fused into the
    scalar-engine activation (per-partition scale folded into weights on host).
  - Attention logits A^T = Wa^T @ h4T, softmax over time on the free axis.
  - DMA-xbar transposes bring As and h4 into time-on-partition layout for the
    pooled = As^T @ h4 and G = As^T @ As contractions over time.
  - Per-segment x mean/sumsq stats on the vector engine.
Host does the tiny [64 x 4608] head + log_softmax + penalty reduction in f64.
"""
from contextlib import ExitStack

import numpy as np
import ml_dtypes

N, D, H, R, O, B, L = 65536, 256, 512, 8, 4, 64, 1024
EPS = 1e-5
NCORES = 8
SEG = B // NCORES          # segments per core
ROWS = SEG * L             # rows per core
BF = ml_dtypes.bfloat16

TRACE = False              # test harness may flip this for profiling
LAST_RESULT = None         # BassKernelResults of the last run (when TRACE)

_BUILT = None              # cached (nc, names)


def _build_nc():
    import concourse.bass as bass
    import concourse.mybir as mybir
    import concourse.tile as tile
    from concourse import bacc
    from concourse.bass import ts

    dt = mybir.dt
    f32 = dt.float32
    bf16 = dt.bfloat16
    Alu = mybir.AluOpType
    Act = mybir.ActivationFunctionType

    nc = bacc.Bacc("TRN2", target_bir_lowering=False, debug=False)

    xT_d = nc.dram_tensor("xT", [128, 2, ROWS], bf16, kind="ExternalInput")
    w1_d = nc.dram_tensor("w1", [128, 2, 512], bf16, kind="ExternalInput")
    w2_d = nc.dram_tensor("w2", [128, 4, 512], bf16, kind="ExternalInput")
    w3_d = nc.dram_tensor("w3", [128, 4, 512], bf16, kind="ExternalInput")
    w4_d = nc.dram_tensor("w4", [128, 4, 512], bf16, kind="ExternalInput")
    wa_d = nc.dram_tensor("wa", [128, 4, 8], bf16, kind="ExternalInput")
    bias_d = nc.dram_tensor("bias", [128, 4, 4], f32, kind="ExternalInput")

    pooled_d = nc.dram_tensor("pooled", [SEG, 8, 512], f32, kind="ExternalOutput")
    g_d = nc.dram_tensor("gmat", [SEG, 8, 8], f32, kind="ExternalOutput")
    xsum_d = nc.dram_tensor("xsum", [128, 2, SEG], f32, kind="ExternalOutput")
    xsq_d = nc.dram_tensor("xsq", [128, 2, SEG], f32, kind="ExternalOutput")

    xT = xT_d.ap()

    with tile.TileContext(nc) as tc, ExitStack() as ctx:
        wpool = ctx.enter_context(tc.tile_pool(name="w", bufs=1))
        xpool = ctx.enter_context(tc.tile_pool(name="x", bufs=2))
        hpools = [ctx.enter_context(tc.tile_pool(name=f"h{i}", bufs=2))
                  for i in range(1, 5)]
        h4npool = ctx.enter_context(tc.tile_pool(name="h4n", bufs=2))
        aspool = ctx.enter_context(tc.tile_pool(name="ast", bufs=2))
        smpool = ctx.enter_context(tc.tile_pool(name="sm", bufs=2))
        scrpool = ctx.enter_context(tc.tile_pool(name="scr", bufs=2))
        stpool = ctx.enter_context(tc.tile_pool(name="st", bufs=2))
        statpool = ctx.enter_context(tc.tile_pool(name="stat", bufs=1))
        pmpool = ctx.enter_context(
            tc.tile_pool(name="pm", bufs=4, space=bass.MemorySpace.PSUM))
        papool = ctx.enter_context(
            tc.tile_pool(name="pa", bufs=2, space=bass.MemorySpace.PSUM))
        pppool = ctx.enter_context(
            tc.tile_pool(name="pp", bufs=1, space=bass.MemorySpace.PSUM))
        pgpool = ctx.enter_context(
            tc.tile_pool(name="pg", bufs=1, space=bass.MemorySpace.PSUM))

        # --- load first-layer weights before anything else ---
        w1 = wpool.tile([128, 2, 512], bf16)
        nc.gpsimd.dma_start(w1[:], w1_d.ap()[:])
        bias = wpool.tile([128, 4, 4], f32)
        nc.gpsimd.dma_start(bias[:], bias_d.ap()[:])
        ws = [w1]
        wa = None

        xsum_acc = statpool.tile([128, 2, SEG], f32)
        xsq_acc = statpool.tile([128, 2, SEG], f32)

        pending = None  # pooled/G emission deferred from previous segment

        for s in range(SEG):
            # --- input tile ---
            xt = xpool.tile([128, 2, L], bf16)
            nc.gpsimd.dma_start(xt[:], xT[:, :, ts(s, L)])

            if s == 0:
                # remaining weights load behind the first input tile
                for wd in (w2_d, w3_d, w4_d):
                    w = wpool.tile([128, 4, 512], bf16, tag=wd.name)
                    nc.gpsimd.dma_start(w[:], wd.ap()[:])
                    ws.append(w)
                wa = wpool.tile([128, 4, 8], bf16)
                nc.gpsimd.dma_start(wa[:], wa_d.ap()[:])

            # --- x stats (DVE) ---
            for ch in range(2):
                nc.vector.reduce_sum(xsum_acc[:, ch, s:s + 1], xt[:, ch, :],
                                     axis=mybir.AxisListType.X)
                scr = scrpool.tile([128, L], bf16)
                nc.scalar.activation(scr[:], xt[:, ch, :], Act.Square,
                                     accum_out=xsq_acc[:, ch, s:s + 1])

            # --- MLP in transposed layout ---
            prev, prev_k = xt, 2
            h = None
            for li in range(4):
                if li == 2 and pending is not None:
                    # previous segment's pooled/G matmuls land here in the
                    # PE stream, by which time its transposes have finished
                    pending()
                    pending = None
                h = hpools[li].tile([128, 4, L], bf16)
                for tb in range(2):
                    for m in range(4):
                        ps = pmpool.tile([128, 512], f32)
                        for k in range(prev_k):
                            nc.tensor.matmul(
                                ps[:],
                                ws[li][:, k, ts(m, 128)],
                                prev[:, k, ts(tb, 512)],
                                start=(k == 0), stop=(k == prev_k - 1))
                        if li % 2 == 0:
                            # BN+ReLU on DVE: (psum + bias) max 0 -> bf16
                            nc.vector.tensor_scalar(
                                h[:, m, ts(tb, 512)], ps[:],
                                bias[:, li, m:m + 1], 0.0,
                                op0=Alu.add, op1=Alu.max)
                        else:
                            nc.scalar.activation(
                                h[:, m, ts(tb, 512)], ps[:], Act.Relu,
                                bias=bias[:, li, m:m + 1], scale=1.0)
                prev, prev_k = h, 4
            h4 = h

            # --- attention logits + softmax over time (free axis) ---
            amax = smpool.tile([8, 2], f32)
            easT = smpool.tile([8, L], f32)
            zpart = smpool.tile([8, 2], f32)
            pa_tiles = []
            for tb in range(2):
                pa = papool.tile([8, 512], f32)
                for k in range(4):
                    nc.tensor.matmul(pa[:], wa[:, k, :], h4[:, k, ts(tb, 512)],
                                     start=(k == 0), stop=(k == 3))
                nc.vector.reduce_max(amax[:, tb:tb + 1], pa[:],
                                     axis=mybir.AxisListType.X, negate=True)
                pa_tiles.append(pa)
            negmax = smpool.tile([8, 1], f32)
            nc.vector.tensor_tensor(negmax[:], amax[:, 0:1], amax[:, 1:2],
                                    op=Alu.min)
            for tb in range(2):
                nc.scalar.activation(easT[:, ts(tb, 512)], pa_tiles[tb][:],
                                     Act.Exp, bias=negmax[:, 0:1], scale=1.0,
                                     accum_out=zpart[:, tb:tb + 1])
            rz = smpool.tile([8, 1], f32)
            nc.vector.tensor_tensor(rz[:], zpart[:, 0:1], zpart[:, 1:2],
                                    op=Alu.add)
            nc.vector.reciprocal(rz[:], rz[:])
            asT16 = smpool.tile([16, L], bf16)
            nc.vector.memset(asT16[:], 0.0)
            nc.vector.tensor_scalar_mul(asT16[0:8, :], easT[:], rz[:, 0:1])

            # --- transposes to time-on-partition layout (DMA xbar) ---
            # h4 transposes first: h4 is ready before the softmax finishes
            h4n = h4npool.tile([128, 8, 4, 128], bf16)
            for ch in range(4):
                nc.sync.dma_start(h4n[:, :, ch, :], h4[:, ch, :],
                                  transpose=True)
            as_t = aspool.tile([128, 8, 16], bf16)
            nc.sync.dma_start(as_t[:], asT16[:], transpose=True)

            def make_pending(s, as_t, h4n):
                def emit():
                    # pooled and G (contract over time)
                    pp = pppool.tile([8, 512], f32)
                    for tt in range(8):
                        nc.tensor.matmul(pp[:], as_t[:, tt, 0:8],
                                         h4n[:, tt, :, :],
                                         start=(tt == 0), stop=(tt == 7))
                    pg = pgpool.tile([8, 8], f32)
                    for tt in range(8):
                        nc.tensor.matmul(pg[:], as_t[:, tt, 0:8],
                                         as_t[:, tt, 0:8],
                                         start=(tt == 0), stop=(tt == 7))
                    post = stpool.tile([8, 512], f32)
                    nc.scalar.copy(post[:], pp[:])
                    nc.gpsimd.dma_start(pooled_d.ap()[s], post[:])
                    gst = stpool.tile([8, 8], f32)
                    nc.vector.tensor_copy(gst[:], pg[:])
                    nc.gpsimd.dma_start(g_d.ap()[s], gst[:])
                return emit

            pending = make_pending(s, as_t, h4n)

        pending()
        nc.gpsimd.dma_start(xsum_d.ap()[:], xsum_acc[:])
        nc.gpsimd.dma_start(xsq_d.ap()[:], xsq_acc[:])

    nc.compile()
    return nc


def _get_nc():
    global _BUILT
    if _BUILT is None:
        _BUILT = _build_nc()
    return _BUILT


def _fuse_bn(W, b, g, be, m, v):
    scale = (np.asarray(g, np.float32)
             / np.sqrt(np.asarray(v, np.float32) + EPS))
    Wf = np.asarray(W, np.float32) * scale[None, :]
    bias = (np.asarray(b, np.float32) - np.asarray(m, np.float32)) * scale \
        + np.asarray(be, np.float32)
    return Wf, bias


def _chunked(Wf):
    """[K, F] f32 -> [128, K//128, F] bf16 (partition-major k-chunks)."""
    K, F = Wf.shape
    return np.ascontiguousarray(
        Wf.reshape(K // 128, 128, F).transpose(1, 0, 2)).astype(BF)


def kernel(**inputs):
    global LAST_RESULT
    from concourse.bass_utils import run_bass_kernel_spmd

    x = np.asarray(inputs["x"], np.float32)

    biases = []
    wts = []
    for i in range(1, 5):
        Wf, bias = _fuse_bn(inputs[f"W{i}"], inputs[f"b{i}"], inputs[f"g{i}"],
                            inputs[f"be{i}"], inputs[f"m{i}"], inputs[f"v{i}"])
        wts.append(_chunked(Wf))
        biases.append(bias)
    # bias[p, layer, m] = bias_layer[m*128 + p]
    bias_t = np.ascontiguousarray(
        np.stack([b.reshape(4, 128).T for b in biases], axis=1)
    ).astype(np.float32)
    wa_t = _chunked(np.asarray(inputs["Wa"], np.float32))

    shared = {"w1": wts[0], "w2": wts[1], "w3": wts[2], "w4": wts[3],
              "wa": wa_t, "bias": bias_t}

    in_maps = []
    for c in range(NCORES):
        xc = x[c * ROWS:(c + 1) * ROWS, :]                 # [ROWS, 256]
        xct = np.ascontiguousarray(
            xc.T.reshape(2, 128, ROWS).transpose(1, 0, 2)).astype(BF)
        in_maps.append({"xT": xct, **shared})

    nc = _get_nc()
    res = run_bass_kernel_spmd(nc, in_maps, core_ids=list(range(NCORES)),
                               trace=TRACE)
    LAST_RESULT = res

    pooled = np.zeros((B, R * H), np.float64)
    G = np.zeros((B, R, R), np.float64)
    xsum = np.zeros((B, D), np.float64)
    xsq = np.zeros((B, D), np.float64)
    for c, out in enumerate(res.results):
        sl = slice(c * SEG, (c + 1) * SEG)
        pooled[sl] = out["pooled"].astype(np.float64).reshape(SEG, R * H)
        G[sl] = out["gmat"].astype(np.float64)
        # xsum/xsq: [128, 2, SEG] -> [SEG, 256] with d = ch*128 + p
        xs = out["xsum"].astype(np.float64).transpose(2, 1, 0).reshape(SEG, D)
        xq = out["xsq"].astype(np.float64).transpose(2, 1, 0).reshape(SEG, D)
        xsum[sl] = xs
        xsq[sl] = xq

    penalty = np.sum((G - 1.0) ** 2)

    mean = xsum / L
    var = (xsq - L * mean * mean) / (L - 1)
    std = np.sqrt(np.maximum(var, 0.0))
    feat = np.concatenate([pooled, mean, std], axis=1)

    c64 = {k: np.asarray(inputs[k], np.float64)
           for k in ("Wo1", "bo1", "go", "beo", "mo", "vo", "Wo2", "bo2")}
    z = feat @ c64["Wo1"] + c64["bo1"]
    o = np.maximum(
        c64["go"] * (z - c64["mo"]) / np.sqrt(c64["vo"] + EPS) + c64["beo"],
        0.0)
    logits = o @ c64["Wo2"] + c64["bo2"]
    ls = logits - logits.max(axis=1, keepdims=True)
    logp = (ls - np.log(np.exp(ls).sum(axis=1, keepdims=True)))

    return (logp.astype(np.float32), np.float32(penalty))
